# revision 28
# baseline (speedup 1.0000x reference)
"""Sparse-attention (entity_mention_select) Trainium2 kernel.

Per entity b: q = relation_matrix[label_b]; scores = node_b @ q;
masked softmax over nodes; out_b = softmax(scores) @ node_b.

Sharding: pure data parallel over B=512 entities -> 64 per NeuronCore x 8.
"""

import sys

for _p in ("/opt/trn_rl_repo", "/root/.axon_site/_ro/trn_rl_repo"):
    if _p not in sys.path:
        sys.path.append(_p)

import numpy as np
import ml_dtypes
from contextlib import ExitStack

import concourse.tile as tile
from concourse import bacc, mybir
from concourse.bass_utils import run_bass_kernel_spmd

F32 = mybir.dt.float32
F32R = mybir.dt.float32r  # PE full-rate fp32 (tf32-like rounding in PE, ~1.5e-4)
BF16 = mybir.dt.bfloat16
I32 = mybir.dt.int32
# node data in bf16: halves HBM traffic (the memory roofline) and doubles DVE
# throughput for the score pass (2x_1P mode needs 16-bit operands).
NDT = BF16
NP_NDT = ml_dtypes.bfloat16
ALU = mybir.AluOpType
ACTF = mybir.ActivationFunctionType

B, N, D, R = 512, 1024, 256, 100
NCORES = 8
BPC = B // NCORES  # 64 entities per core
NCH = N // 128     # 8 node chunks of 128
GRP_SIZE = 2       # entities per denominator/reciprocal batch


def build_tile_kernel(tc, outs, ins):
    nc = tc.nc
    node = ins["node"]          # [BPC, N, D] f32
    edge_t = ins["edge_t"]      # [128, BPC*NCH] i32  (edge_t[p, b*NCH+c] = edge[b, c*128+p])
    labels = ins["labels"]      # [1, BPC] i32
    relmat = ins["relmat"]      # [R, D] f32
    iota_r = ins["iota"]        # [128, BPC] f32, row r filled with value r
    ones_c = ins["ones_col"]    # [1, 128] f32
    ones_r = ins["ones_row"]    # [128, 1] f32
    out = outs["out"]           # [1, BPC*D] f32

    # node DRAM is [BPC, 128, NCH*D]: per entity a flat [128, 2048] tile where
    # element (p, j*D+d) = node[8p+j, d] — fully contiguous DMA, 2KB/partition.
    # scores/mask use the same (p, j) <-> n = 8p+j mapping.
    GRP = GRP_SIZE  # entities per denominator/reciprocal batch

    with ExitStack() as ctx:
        const_pool = ctx.enter_context(tc.tile_pool(name="const", bufs=1))
        node_pool = ctx.enter_context(tc.tile_pool(name="node", bufs=6))
        qb_pool = ctx.enter_context(tc.tile_pool(name="qb", bufs=4))
        small_pool = ctx.enter_context(tc.tile_pool(name="small", bufs=4))
        scr_pool = ctx.enter_context(tc.tile_pool(name="scr", bufs=3))
        outbuf_pool = ctx.enter_context(tc.tile_pool(name="outb", bufs=1))
        ps_qb = ctx.enter_context(tc.tile_pool(name="ps_qb", bufs=2, space="PSUM"))
        ps_out = ctx.enter_context(tc.tile_pool(name="ps_out", bufs=4, space="PSUM"))
        ps_den = ctx.enter_context(tc.tile_pool(name="ps_den", bufs=1, space="PSUM"))
        ps_setup = ctx.enter_context(tc.tile_pool(name="ps_setup", bufs=1, space="PSUM"))

        # ---------- setup ----------
        relmat_sb = const_pool.tile([128, D], NDT, tag="relmat")
        nc.sync.dma_start(relmat_sb[:R, :], relmat[:, :])
        mask_sb = const_pool.tile([128, BPC * NCH], F32, tag="mask")
        nc.gpsimd.dma_start(mask_sb[:], edge_t[:, :])  # i32 -> f32 cast
        labels_f = const_pool.tile([1, BPC], F32, tag="labels")
        nc.gpsimd.dma_start(labels_f[:], labels[:, :])  # i32 -> f32 cast
        iota_sb = const_pool.tile([128, BPC], F32, tag="iota")
        nc.sync.dma_start(iota_sb[:], iota_r[:, :])
        ones_c_sb = const_pool.tile([1, 128], F32, tag="ones_c")
        nc.sync.dma_start(ones_c_sb[:], ones_c[:, :])
        ones_r_sb = const_pool.tile([128, 1], F32, tag="ones_r")
        nc.sync.dma_start(ones_r_sb[:], ones_r[:, :])

        # labels broadcast to R partitions, then one-hot^T[r, b] = (label_b == r)
        lab_ps = ps_setup.tile([R, BPC], F32, tag="lab")
        nc.tensor.matmul(lab_ps[:], ones_c_sb[:1, :R], labels_f[:1, :], start=True, stop=True)
        onehotT = const_pool.tile([128, BPC], NDT, tag="onehot")
        nc.vector.tensor_tensor(onehotT[:R, :], lab_ps[:R, :], iota_sb[:R, :], ALU.is_equal)

        out_sb = outbuf_pool.tile([1, BPC * D], F32, tag="out")
        neg30 = const_pool.tile([128, 1], F32, tag="neg30")
        nc.gpsimd.memset(neg30[:], -30.0)

        # ---------- per-entity pipeline ----------
        node_sbs = {}
        for g in range(BPC // GRP):
            grp = range(g * GRP, (g + 1) * GRP)
            esums = small_pool.tile([128, GRP], F32, tag="esums")
            o_pss = []
            for gi, b in enumerate(grp):
                # one 1 MB DMA covers a pair of entities; DRAM is laid out
                # pair-major so each partition is a single contiguous 8KB run
                if b % 2 == 0:
                    pair_sb = node_pool.tile([128, 2 * NCH * D], NDT, tag="node")
                    dma_eng = nc.sync if (b // 2) % 2 == 0 else nc.scalar
                    dma_eng.dma_start(pair_sb[:], node[b // 2])
                    node_sbs[b] = pair_sb[:, : NCH * D]
                    node_sbs[b + 1] = pair_sb[:, NCH * D :]
                node_sb = node_sbs[b]

                # q_b broadcast to 128 partitions: onehot col (bcast) @ relmat
                qb_ps = ps_qb.tile([128, D], F32, tag="qb")
                nc.tensor.matmul(
                    qb_ps[:],
                    onehotT[:R, b : b + 1].broadcast_to((R, 128)),
                    relmat_sb[:R, :],
                    start=True,
                    stop=True,
                )
                qb_sb = qb_pool.tile([128, D], NDT, tag="qbs")
                nc.scalar.copy(qb_sb[:], qb_ps[:])

                # scores[p, j] = sum_d node[8p+j, d] * q[d]
                scores = small_pool.tile([128, NCH], F32, tag="scores")
                for c in range(NCH):
                    scr = scr_pool.tile([128, D], NDT, tag="scr")
                    nc.vector.scalar_tensor_tensor(
                        scr[:],
                        node_sb[:, c * D : (c + 1) * D],
                        1.0,
                        qb_sb[:],
                        ALU.mult,
                        ALU.mult,
                        accum_out=scores[:, c : c + 1],
                    )

                # masked softmax numerator without a DVE->ACT->DVE chain:
                # sm = (scores+30)*mask on DVE, then em = exp(sm-30) on ACT
                # (masked slots -> exp(-30) ~ 9e-14, negligible in the sums);
                # ACT accum_out gives the per-partition row sums for free.
                sm_sb = small_pool.tile([128, NCH], F32, tag="sm")
                nc.vector.scalar_tensor_tensor(
                    sm_sb[:],
                    scores[:],
                    30.0,
                    mask_sb[:, b * NCH : (b + 1) * NCH],
                    ALU.add,
                    ALU.mult,
                )
                em_sb = small_pool.tile([128, NCH], NDT, tag="em")
                nc.scalar.activation(
                    em_sb[:],
                    sm_sb[:],
                    ACTF.Exp,
                    bias=neg30[:],
                    accum_out=esums[:, gi : gi + 1],
                )

                # out_raw[d] = sum_n w[n] * node[n, d]  (unnormalized weights)
                o_ps = ps_out.tile([1, D], F32, tag="oraw")
                for c in range(NCH):
                    nc.tensor.matmul(
                        o_ps[:],
                        em_sb[:, c : c + 1],
                        node_sb[:, c * D : (c + 1) * D],
                        start=(c == 0),
                        stop=(c == NCH - 1),
                    )
                o_pss.append(o_ps)

            # batched denominator + reciprocal for the group
            den_ps = ps_den.tile([1, GRP], F32, tag="den")
            nc.tensor.matmul(den_ps[:], ones_r_sb[:], esums[:], start=True, stop=True)
            recip = small_pool.tile([1, GRP], F32, tag="recip")
            nc.vector.reciprocal(recip[:], den_ps[:])
            for gi, b in enumerate(grp):
                nc.scalar.activation(
                    out_sb[:1, b * D : (b + 1) * D],
                    o_pss[gi][:],
                    ACTF.Copy,
                    scale=recip[:1, gi : gi + 1],
                )

        nc.sync.dma_start(out[:, :], out_sb[:])


# ---------------------------------------------------------------------------
# host-side driver
# ---------------------------------------------------------------------------

_CACHE = {}


def _constants():
    iota = np.broadcast_to(np.arange(128, dtype=np.float32)[:, None], (128, BPC)).copy()
    ones_col = np.ones((1, 128), np.float32)
    ones_row = np.ones((128, 1), np.float32)
    return iota, ones_col, ones_row


def declare_io(nc):
    ins = {
        "node": nc.dram_tensor("node", [BPC // 2, 128, 2 * NCH * D], NDT, kind="ExternalInput").ap(),
        "edge_t": nc.dram_tensor("edge_t", [128, BPC * NCH], I32, kind="ExternalInput").ap(),
        "labels": nc.dram_tensor("labels", [1, BPC], I32, kind="ExternalInput").ap(),
        "relmat": nc.dram_tensor("relmat", [R, D], NDT, kind="ExternalInput").ap(),
        "iota": nc.dram_tensor("iota", [128, BPC], F32, kind="ExternalInput").ap(),
        "ones_col": nc.dram_tensor("ones_col", [1, 128], F32, kind="ExternalInput").ap(),
        "ones_row": nc.dram_tensor("ones_row", [128, 1], F32, kind="ExternalInput").ap(),
    }
    outs = {"out": nc.dram_tensor("out", [1, BPC * D], F32, kind="ExternalOutput").ap()}
    return ins, outs


def _build_nc():
    if "nc" in _CACHE:
        return _CACHE["nc"]
    nc = bacc.Bacc(
        "TRN2",
        target_bir_lowering=False,
        debug=False,
        enable_asserts=False,
        num_devices=NCORES,
    )
    ins, outs = declare_io(nc)
    with tile.TileContext(nc) as tc:
        build_tile_kernel(tc, outs, ins)
    nc.compile()
    _CACHE["nc"] = nc
    return nc


def make_in_maps(node_feature, edge_weight, relation_label, relation_matrix):
    iota, ones_col, ones_row = _constants()
    relmat = np.ascontiguousarray(np.asarray(relation_matrix, dtype=np.float32).astype(NP_NDT))
    node_f32 = np.asarray(node_feature, dtype=np.float32)
    in_maps = []
    for core in range(NCORES):
        sl = slice(core * BPC, (core + 1) * BPC)
        # pair-major layout: node_c[pair, p, e*2048+f] = per-entity tile
        # element (p, f) of entity 2*pair+e, so each DMA partition row is
        # one contiguous 8KB run in DRAM
        node_c = np.ascontiguousarray(
            node_f32[sl]
            .astype(NP_NDT)
            .reshape(BPC // 2, 2, 128, NCH * D)
            .transpose(0, 2, 1, 3)
            .reshape(BPC // 2, 128, 2 * NCH * D)
        )
        edge_c = np.asarray(edge_weight[sl], dtype=np.int32)
        # edge_t[p, b*NCH + j] = edge[b, 8*p + j]  (matches node tile layout)
        edge_t = np.ascontiguousarray(
            edge_c.reshape(BPC, 128, NCH).transpose(1, 0, 2).reshape(128, BPC * NCH)
        )
        labels_c = np.ascontiguousarray(
            np.asarray(relation_label[sl], dtype=np.int32).reshape(1, BPC)
        )
        in_maps.append(
            {
                "node": node_c,
                "edge_t": edge_t,
                "labels": labels_c,
                "relmat": relmat,
                "iota": iota,
                "ones_col": ones_col,
                "ones_row": ones_row,
            }
        )
    return in_maps


def run(node_feature, edge_weight, relation_label, relation_matrix, trace=False):
    nc = _build_nc()
    in_maps = make_in_maps(node_feature, edge_weight, relation_label, relation_matrix)
    res = run_bass_kernel_spmd(nc, in_maps, core_ids=list(range(NCORES)), trace=trace)
    out = np.concatenate(
        [res.results[c]["out"].reshape(BPC, D) for c in range(NCORES)], axis=0
    )
    return out.astype(np.float32), res


def kernel(node_feature, edge_weight, relation_label, relation_matrix):
    out, _ = run(node_feature, edge_weight, relation_label, relation_matrix)
    return out


# ---------------------------------------------------------------------------
# wall-clock timing helper (no NTFF profiling available under this axon setup)
# ---------------------------------------------------------------------------


def make_timed_runner(nc, in_maps):
    """Build a jitted 8-core runner with inputs resident on device.

    Returns (call, out_names): `call()` executes once, blocking, and returns
    the jax output arrays. Mirrors bass2jax.run_bass_via_pjrt's multi-core
    branch, but keeps the big inputs on device across calls so repeated calls
    time [dispatch + kernel exec] only.
    """
    import jax
    from jax.sharding import Mesh, PartitionSpec
    from jax.experimental.shard_map import shard_map
    from concourse import bass2jax as b2j
    from concourse import mybir as _mb

    b2j.install_neuronx_cc_hook()
    n_cores = len(in_maps)

    partition_name = nc.partition_id_tensor.name if nc.partition_id_tensor else None
    in_names, out_names, out_avals, zero_outs = [], [], [], []
    for alloc in nc.m.functions[0].allocations:
        if not isinstance(alloc, _mb.MemoryLocationSet):
            continue
        name = alloc.memorylocations[0].name
        if alloc.kind == "ExternalInput":
            if name != partition_name:
                in_names.append(name)
        elif alloc.kind == "ExternalOutput":
            out_names.append(name)
            shape = tuple(alloc.tensor_shape)
            dtype = _mb.dt.np(alloc.dtype)
            out_avals.append(jax.core.ShapedArray(shape, dtype))
            zero_outs.append(np.zeros(shape, dtype))
    n_params = len(in_names)
    all_in_names = in_names + out_names
    if partition_name is not None:
        all_in_names.append(partition_name)

    def _body(*args):
        operands = list(args)
        if partition_name is not None:
            operands.append(b2j.partition_id_tensor())
        outs = b2j._bass_exec_p.bind(
            *operands,
            out_avals=tuple(out_avals),
            in_names=tuple(all_in_names),
            out_names=tuple(out_names),
            lowering_input_output_aliases=(),
            sim_require_finite=True,
            sim_require_nnan=True,
            nc=nc,
        )
        return tuple(outs)

    devices = jax.devices()[:n_cores]
    mesh = Mesh(np.asarray(devices), ("core",))
    in_specs = (PartitionSpec("core"),) * (n_params + len(out_names))
    out_specs = (PartitionSpec("core"),) * len(out_names)
    donate = tuple(range(n_params, n_params + len(out_names)))
    sharded = jax.jit(
        shard_map(
            _body, mesh=mesh, in_specs=in_specs, out_specs=out_specs, check_rep=False
        ),
        donate_argnums=donate,
        keep_unused=True,
    )

    sharding = jax.sharding.NamedSharding(mesh, PartitionSpec("core"))
    dev_in = [
        jax.device_put(
            np.concatenate([np.asarray(m[name]) for m in in_maps], axis=0), sharding
        )
        for name in in_names
    ]

    def call():
        zeros = [np.zeros((n_cores * z.shape[0], *z.shape[1:]), z.dtype) for z in zero_outs]
        outs = sharded(*dev_in, *zeros)
        jax.block_until_ready(outs)
        return outs

    return call, out_names



# revision 37
# speedup vs baseline: 1.7248x; 1.7248x over previous
"""Sparse-attention (entity_mention_select) Trainium2 kernel.

Per entity b: q = relation_matrix[label_b]; scores = node_b @ q;
masked softmax over nodes; out_b = softmax(scores) @ node_b.

Sharding: pure data parallel over B=512 entities -> 64 per NeuronCore x 8.
"""

import sys

for _p in ("/opt/trn_rl_repo", "/root/.axon_site/_ro/trn_rl_repo"):
    if _p not in sys.path:
        sys.path.append(_p)

import numpy as np
import ml_dtypes
from contextlib import ExitStack

import concourse.tile as tile
from concourse import bacc, mybir
from concourse.bass_utils import run_bass_kernel_spmd

F32 = mybir.dt.float32
F32R = mybir.dt.float32r  # PE full-rate fp32 (tf32-like rounding in PE, ~1.5e-4)
BF16 = mybir.dt.bfloat16
I32 = mybir.dt.int32
# node data in bf16: halves HBM traffic (the memory roofline) and doubles DVE
# throughput for the score pass (2x_1P mode needs 16-bit operands).
NDT = BF16
NP_NDT = ml_dtypes.bfloat16
ALU = mybir.AluOpType
ACTF = mybir.ActivationFunctionType

B, N, D, R = 512, 1024, 256, 100
NCORES = 8
BPC = B // NCORES  # 64 entities per core
NCH = N // 128     # 8 node chunks of 128
GRP_SIZE = 2       # entities per denominator/reciprocal batch


def build_tile_kernel(tc, outs, ins):
    nc = tc.nc
    node = ins["node"]          # [BPC, N, D] f32
    edge_t = ins["edge_t"]      # [128, BPC*NCH] i32  (edge_t[p, b*NCH+c] = edge[b, c*128+p])
    labels = ins["labels"]      # [1, BPC] i32
    relmat = ins["relmat"]      # [R, D] f32
    iota_r = ins["iota"]        # [128, BPC] f32, row r filled with value r
    ones_c = ins["ones_col"]    # [1, 128] f32
    ones_r = ins["ones_row"]    # [128, 1] f32
    out = outs["out"]           # [1, BPC*D] f32

    # node DRAM is [BPC, 128, NCH*D]: per entity a flat [128, 2048] tile where
    # element (p, j*D+d) = node[8p+j, d] — fully contiguous DMA, 2KB/partition.
    # scores/mask use the same (p, j) <-> n = 8p+j mapping.
    GRP = GRP_SIZE  # entities per denominator/reciprocal batch

    with ExitStack() as ctx:
        const_pool = ctx.enter_context(tc.tile_pool(name="const", bufs=1))
        node_pool = ctx.enter_context(tc.tile_pool(name="node", bufs=8))
        qb_pool = ctx.enter_context(tc.tile_pool(name="qb", bufs=4))
        small_pool = ctx.enter_context(tc.tile_pool(name="small", bufs=4))
        scr_pool = ctx.enter_context(tc.tile_pool(name="scr", bufs=3))
        outbuf_pool = ctx.enter_context(tc.tile_pool(name="outb", bufs=1))
        ps_qb = ctx.enter_context(tc.tile_pool(name="ps_qb", bufs=2, space="PSUM"))
        ps_out = ctx.enter_context(tc.tile_pool(name="ps_out", bufs=4, space="PSUM"))
        ps_den = ctx.enter_context(tc.tile_pool(name="ps_den", bufs=1, space="PSUM"))
        ps_setup = ctx.enter_context(tc.tile_pool(name="ps_setup", bufs=1, space="PSUM"))

        # ---------- setup ----------
        relmat_sb = const_pool.tile([128, D], NDT, tag="relmat")
        nc.sync.dma_start(relmat_sb[:R, :], relmat[:, :])
        mask_sb = const_pool.tile([128, BPC * NCH], F32, tag="mask")
        nc.gpsimd.dma_start(mask_sb[:], edge_t[:, :])  # i32 -> f32 cast
        labels_f = const_pool.tile([1, BPC], F32, tag="labels")
        nc.gpsimd.dma_start(labels_f[:], labels[:, :])  # i32 -> f32 cast
        iota_sb = const_pool.tile([128, BPC], F32, tag="iota")
        nc.sync.dma_start(iota_sb[:], iota_r[:, :])
        ones_c_sb = const_pool.tile([1, 128], F32, tag="ones_c")
        nc.sync.dma_start(ones_c_sb[:], ones_c[:, :])
        ones_r_sb = const_pool.tile([128, 1], F32, tag="ones_r")
        nc.sync.dma_start(ones_r_sb[:], ones_r[:, :])

        # labels broadcast to R partitions, then one-hot^T[r, b] = (label_b == r)
        lab_ps = ps_setup.tile([R, BPC], F32, tag="lab")
        nc.tensor.matmul(lab_ps[:], ones_c_sb[:1, :R], labels_f[:1, :], start=True, stop=True)
        onehotT = const_pool.tile([128, BPC], NDT, tag="onehot")
        nc.vector.tensor_tensor(onehotT[:R, :], lab_ps[:R, :], iota_sb[:R, :], ALU.is_equal)

        out_sb = outbuf_pool.tile([1, BPC * D], F32, tag="out")
        neg30 = const_pool.tile([128, 1], F32, tag="neg30")
        nc.gpsimd.memset(neg30[:], -30.0)

        # ---------- per-entity pipeline ----------
        node_sbs = {}
        for g in range(BPC // GRP):
            grp = range(g * GRP, (g + 1) * GRP)
            esums = small_pool.tile([128, GRP], F32, tag="esums")
            o_pss = []
            for gi, b in enumerate(grp):
                # one 1 MB DMA covers a pair of entities; DRAM is laid out
                # pair-major so each partition is a single contiguous 8KB run
                if b % 2 == 0:
                    pair_sb = node_pool.tile([128, 2 * NCH * D], NDT, tag="node")
                    dma_eng = nc.sync if (b // 2) % 2 == 0 else nc.scalar
                    dma_eng.dma_start(pair_sb[:], node[b // 2])
                    node_sbs[b] = pair_sb[:, : NCH * D]
                    node_sbs[b + 1] = pair_sb[:, NCH * D :]
                node_sb = node_sbs[b]

                # q_b broadcast to 128 partitions: onehot col (bcast) @ relmat
                qb_ps = ps_qb.tile([128, D], F32, tag="qb")
                nc.tensor.matmul(
                    qb_ps[:],
                    onehotT[:R, b : b + 1].broadcast_to((R, 128)),
                    relmat_sb[:R, :],
                    start=True,
                    stop=True,
                )
                qb_sb = qb_pool.tile([128, D], NDT, tag="qbs")
                nc.scalar.copy(qb_sb[:], qb_ps[:])

                # scores[p, j] = sum_d node[8p+j, d] * q[d]
                scores = small_pool.tile([128, NCH], F32, tag="scores")
                for c in range(NCH):
                    scr = scr_pool.tile([128, D], NDT, tag="scr")
                    nc.vector.scalar_tensor_tensor(
                        scr[:],
                        node_sb[:, c * D : (c + 1) * D],
                        1.0,
                        qb_sb[:],
                        ALU.mult,
                        ALU.mult,
                        accum_out=scores[:, c : c + 1],
                    )

                # masked softmax numerator without a DVE->ACT->DVE chain:
                # sm = (scores+30)*mask on DVE, then em = exp(sm-30) on ACT
                # (masked slots -> exp(-30) ~ 9e-14, negligible in the sums);
                # ACT accum_out gives the per-partition row sums for free.
                sm_sb = small_pool.tile([128, NCH], F32, tag="sm")
                nc.vector.scalar_tensor_tensor(
                    sm_sb[:],
                    scores[:],
                    30.0,
                    mask_sb[:, b * NCH : (b + 1) * NCH],
                    ALU.add,
                    ALU.mult,
                )
                em_sb = small_pool.tile([128, NCH], NDT, tag="em")
                nc.scalar.activation(
                    em_sb[:],
                    sm_sb[:],
                    ACTF.Exp,
                    bias=neg30[:],
                    accum_out=esums[:, gi : gi + 1],
                )

                # out_raw[d] = sum_n w[n] * node[n, d]  (unnormalized weights)
                o_ps = ps_out.tile([1, D], F32, tag="oraw")
                for c in range(NCH):
                    nc.tensor.matmul(
                        o_ps[:],
                        em_sb[:, c : c + 1],
                        node_sb[:, c * D : (c + 1) * D],
                        start=(c == 0),
                        stop=(c == NCH - 1),
                    )
                o_pss.append(o_ps)

            # batched denominator + reciprocal for the group
            den_ps = ps_den.tile([1, GRP], F32, tag="den")
            nc.tensor.matmul(den_ps[:], ones_r_sb[:], esums[:], start=True, stop=True)
            recip = small_pool.tile([1, GRP], F32, tag="recip")
            nc.vector.reciprocal(recip[:], den_ps[:])
            for gi, b in enumerate(grp):
                nc.scalar.activation(
                    out_sb[:1, b * D : (b + 1) * D],
                    o_pss[gi][:],
                    ACTF.Copy,
                    scale=recip[:1, gi : gi + 1],
                )

        nc.sync.dma_start(out[:, :], out_sb[:])


# ---------------------------------------------------------------------------
# host-side driver
# ---------------------------------------------------------------------------

_CACHE = {}


def _constants():
    iota = np.broadcast_to(np.arange(128, dtype=np.float32)[:, None], (128, BPC)).copy()
    ones_col = np.ones((1, 128), np.float32)
    ones_row = np.ones((128, 1), np.float32)
    return iota, ones_col, ones_row


def declare_io(nc):
    ins = {
        "node": nc.dram_tensor("node", [BPC // 2, 128, 2 * NCH * D], NDT, kind="ExternalInput").ap(),
        "edge_t": nc.dram_tensor("edge_t", [128, BPC * NCH], I32, kind="ExternalInput").ap(),
        "labels": nc.dram_tensor("labels", [1, BPC], I32, kind="ExternalInput").ap(),
        "relmat": nc.dram_tensor("relmat", [R, D], NDT, kind="ExternalInput").ap(),
        "iota": nc.dram_tensor("iota", [128, BPC], F32, kind="ExternalInput").ap(),
        "ones_col": nc.dram_tensor("ones_col", [1, 128], F32, kind="ExternalInput").ap(),
        "ones_row": nc.dram_tensor("ones_row", [128, 1], F32, kind="ExternalInput").ap(),
    }
    outs = {"out": nc.dram_tensor("out", [1, BPC * D], F32, kind="ExternalOutput").ap()}
    return ins, outs


def _build_nc():
    if "nc" in _CACHE:
        return _CACHE["nc"]
    nc = bacc.Bacc(
        "TRN2",
        target_bir_lowering=False,
        debug=False,
        enable_asserts=False,
        num_devices=NCORES,
    )
    ins, outs = declare_io(nc)
    with tile.TileContext(nc) as tc:
        build_tile_kernel(tc, outs, ins)
    nc.compile()
    _CACHE["nc"] = nc
    return nc


def make_in_maps(node_feature, edge_weight, relation_label, relation_matrix):
    iota, ones_col, ones_row = _constants()
    relmat = np.ascontiguousarray(np.asarray(relation_matrix, dtype=np.float32).astype(NP_NDT))
    node_f32 = np.asarray(node_feature, dtype=np.float32)
    in_maps = []
    for core in range(NCORES):
        sl = slice(core * BPC, (core + 1) * BPC)
        # pair-major layout: node_c[pair, p, e*2048+f] = per-entity tile
        # element (p, f) of entity 2*pair+e, so each DMA partition row is
        # one contiguous 8KB run in DRAM
        node_c = np.ascontiguousarray(
            node_f32[sl]
            .astype(NP_NDT)
            .reshape(BPC // 2, 2, 128, NCH * D)
            .transpose(0, 2, 1, 3)
            .reshape(BPC // 2, 128, 2 * NCH * D)
        )
        edge_c = np.asarray(edge_weight[sl], dtype=np.int32)
        # edge_t[p, b*NCH + j] = edge[b, 8*p + j]  (matches node tile layout)
        edge_t = np.ascontiguousarray(
            edge_c.reshape(BPC, 128, NCH).transpose(1, 0, 2).reshape(128, BPC * NCH)
        )
        labels_c = np.ascontiguousarray(
            np.asarray(relation_label[sl], dtype=np.int32).reshape(1, BPC)
        )
        in_maps.append(
            {
                "node": node_c,
                "edge_t": edge_t,
                "labels": labels_c,
                "relmat": relmat,
                "iota": iota,
                "ones_col": ones_col,
                "ones_row": ones_row,
            }
        )
    return in_maps


def run(node_feature, edge_weight, relation_label, relation_matrix, trace=False):
    nc = _build_nc()
    in_maps = make_in_maps(node_feature, edge_weight, relation_label, relation_matrix)
    res = run_bass_kernel_spmd(nc, in_maps, core_ids=list(range(NCORES)), trace=trace)
    out = np.concatenate(
        [res.results[c]["out"].reshape(BPC, D) for c in range(NCORES)], axis=0
    )
    return out.astype(np.float32), res


def kernel(node_feature, edge_weight, relation_label, relation_matrix):
    out, _ = run(node_feature, edge_weight, relation_label, relation_matrix)
    return out


# ---------------------------------------------------------------------------
# wall-clock timing helper (no NTFF profiling available under this axon setup)
# ---------------------------------------------------------------------------


def make_timed_runner(nc, in_maps):
    """Build a jitted 8-core runner with inputs resident on device.

    Returns (call, out_names): `call()` executes once, blocking, and returns
    the jax output arrays. Mirrors bass2jax.run_bass_via_pjrt's multi-core
    branch, but keeps the big inputs on device across calls so repeated calls
    time [dispatch + kernel exec] only.
    """
    import jax
    from jax.sharding import Mesh, PartitionSpec
    from jax.experimental.shard_map import shard_map
    from concourse import bass2jax as b2j
    from concourse import mybir as _mb

    b2j.install_neuronx_cc_hook()
    n_cores = len(in_maps)

    partition_name = nc.partition_id_tensor.name if nc.partition_id_tensor else None
    in_names, out_names, out_avals, zero_outs = [], [], [], []
    for alloc in nc.m.functions[0].allocations:
        if not isinstance(alloc, _mb.MemoryLocationSet):
            continue
        name = alloc.memorylocations[0].name
        if alloc.kind == "ExternalInput":
            if name != partition_name:
                in_names.append(name)
        elif alloc.kind == "ExternalOutput":
            out_names.append(name)
            shape = tuple(alloc.tensor_shape)
            dtype = _mb.dt.np(alloc.dtype)
            out_avals.append(jax.core.ShapedArray(shape, dtype))
            zero_outs.append(np.zeros(shape, dtype))
    n_params = len(in_names)
    all_in_names = in_names + out_names
    if partition_name is not None:
        all_in_names.append(partition_name)

    def _body(*args):
        operands = list(args)
        if partition_name is not None:
            operands.append(b2j.partition_id_tensor())
        outs = b2j._bass_exec_p.bind(
            *operands,
            out_avals=tuple(out_avals),
            in_names=tuple(all_in_names),
            out_names=tuple(out_names),
            lowering_input_output_aliases=(),
            sim_require_finite=True,
            sim_require_nnan=True,
            nc=nc,
        )
        return tuple(outs)

    devices = jax.devices()[:n_cores]
    mesh = Mesh(np.asarray(devices), ("core",))
    in_specs = (PartitionSpec("core"),) * (n_params + len(out_names))
    out_specs = (PartitionSpec("core"),) * len(out_names)
    donate = tuple(range(n_params, n_params + len(out_names)))
    sharded = jax.jit(
        shard_map(
            _body, mesh=mesh, in_specs=in_specs, out_specs=out_specs, check_rep=False
        ),
        donate_argnums=donate,
        keep_unused=True,
    )

    sharding = jax.sharding.NamedSharding(mesh, PartitionSpec("core"))
    dev_in = [
        jax.device_put(
            np.concatenate([np.asarray(m[name]) for m in in_maps], axis=0), sharding
        )
        for name in in_names
    ]

    def call():
        zeros = [np.zeros((n_cores * z.shape[0], *z.shape[1:]), z.dtype) for z in zero_outs]
        outs = sharded(*dev_in, *zeros)
        jax.block_until_ready(outs)
        return outs

    return call, out_names

